# revision 1
# baseline (speedup 1.0000x reference)
"""Trainium2 Bass kernel for nn_CombinedLoss (Poisson + 3-way pairwise CLIP loss).

Strategy (8 NeuronCores, SPMD, no collectives):
  - Row-shard the batch: core c owns rows [c*512, (c+1)*512) of every tensor.
  - For each feature pair (a,b) in {(1,2),(1,3),(2,3)} each core computes its
    512x4096 block of S_ab = Za @ Zb^T with bf16 matmuls (fp32 PSUM accum):
      lhsT = raw-cast bf16 own-slice of a, transposed on-chip (PE transpose);
      rhs  = normalized bf16 full feature b, transposed via DMA xbar
             (bf16 roundtrip through a DRAM scratch buffer).
    The 1/||a|| normalization of the lhsT side is folded into the exp's
    per-partition scale on the Scalar engine: exp(S_raw * (2/||a_m||)).
  - Row-wise sum(exp) comes free via the activation's accum_out.
  - Column-wise sum(exp) via ones-vector matmuls (contraction over partitions),
    accumulated in PSUM across the 4 M-tiles; partial per-core, host combines.
  - 1/sqrt on device via bit-trick + 2 Newton steps on the Vector engine
    (avoids ACT Ln/Exp table thrashing; ACT does only Exp + poisson Ln).
  - Diagonal similarities via fused multiply+reduce on own slices (raw dots,
    normalized on host with the device-computed squared norms).
  - Host does only the O(B) final combine: log of 4096-length sums, means.
"""

import os
import sys

import numpy as np

sys.path.insert(0, "/opt/trn_rl_repo")

P = 128
TEMPERATURE = 0.5
EPS_POISSON = 1e-8
RSQRT_MAGIC = 0x5F3759DF


class Cfg:
    def __init__(self, B=4096, D=1024, n_cores=8, ntc=512):
        self.B = B          # batch
        self.D = D          # feature dim
        self.n_cores = n_cores
        self.S = B // n_cores      # own rows per core
        self.MT = self.S // P      # M tiles (own rows / 128)
        self.K = D // P            # contraction tiles
        self.NTC = ntc             # columns per rhs tile
        self.NT = B // ntc         # number of rhs tiles
        self.ST = ntc // P         # row-subtiles per rhs tile
        assert B % n_cores == 0 and self.S % P == 0 and D % P == 0 and B % ntc == 0


def _patch_act_tables():
    """Make Bacc's act-table pass pick `natural_log_exp_and_others` for both
    Exp and Ln (they otherwise land in two different sets, and alternating
    Ln/Exp calls reload the 2.7us activation tables every tile).

    Keeps list order (index == act_func_set_id) but empties the earlier
    exp-only / ln-only sets so the first set containing Exp or Ln is the
    combined one."""
    import functools

    import concourse.hw_specs as hw_specs

    if getattr(hw_specs, "_act_tables_patched", False):
        return
    orig = hw_specs.get_activation_tables

    @functools.cache
    def patched(module_arch):
        tabs = dict(orig(module_arch))
        names = list(tabs.keys())
        if "natural_log_exp_and_others" in tabs:
            combined = tabs["natural_log_exp_and_others"]
            for name in names:
                if name == "natural_log_exp_and_others":
                    break
                if tabs[name] & combined:
                    tabs[name] = tabs[name] - combined
        return tabs

    hw_specs.get_activation_tables = patched
    # bacc imports the symbol lazily via module attr? patch its ref if bound
    import concourse.bacc as bacc_mod

    if hasattr(bacc_mod, "get_activation_tables"):
        bacc_mod.get_activation_tables = patched
    hw_specs._act_tables_patched = True


def build_bass(cfg: Cfg):
    """Build the single-core Bass program (same program for all SPMD cores)."""
    import concourse.bacc as bacc
    import concourse.bass as bass
    import concourse.mybir as mybir
    import concourse.tile as tile
    from concourse.masks import make_identity

    _patch_act_tables()

    f32 = mybir.dt.float32
    bf16 = mybir.dt.bfloat16
    i32 = mybir.dt.int32
    AF = mybir.ActivationFunctionType
    ALU = mybir.AluOpType
    ts = bass.ts

    B, D, K, MT, NT, NTC, ST = cfg.B, cfg.D, cfg.K, cfg.MT, cfg.NT, cfg.NTC, cfg.ST

    nc = bacc.Bacc(
        "TRN2",
        target_bir_lowering=False,
        debug=False,
        enable_asserts=False,
        num_devices=cfg.n_cores,
    )

    # ---- IO ----
    f1o = nc.dram_tensor("f1_own", [cfg.S, D], f32, kind="ExternalInput").ap()
    f2o = nc.dram_tensor("f2_own", [cfg.S, D], f32, kind="ExternalInput").ap()
    f3o = nc.dram_tensor("f3_own", [cfg.S, D], f32, kind="ExternalInput").ap()
    f2f = nc.dram_tensor("f2_full", [B, D], f32, kind="ExternalInput").ap()
    f3f = nc.dram_tensor("f3_full", [B, D], f32, kind="ExternalInput").ap()
    inp = nc.dram_tensor("inp_own", [cfg.S, D], f32, kind="ExternalInput").ap()
    tgt = nc.dram_tensor("tgt_own", [cfg.S, D], f32, kind="ExternalInput").ap()

    rowparts_d = nc.dram_tensor("rowparts", [P, 3 * MT * NT], f32, kind="ExternalOutput").ap()
    colparts_d = nc.dram_tensor("colparts", [1, 3 * B], f32, kind="ExternalOutput").ap()
    nsq_d = nc.dram_tensor("nsq_own", [P, 3 * MT], f32, kind="ExternalOutput").ap()
    dots_d = nc.dram_tensor("dots_own", [P, 3 * MT], f32, kind="ExternalOutput").ap()
    poi_d = nc.dram_tensor("poi", [P, 2 * MT], f32, kind="ExternalOutput").ap()

    own_dram = [f1o, f2o, f3o]

    with tile.TileContext(nc) as tc:
        with (
            tc.tile_pool(name="const", bufs=1) as const_pool,
            tc.tile_pool(name="persist", bufs=1) as persist,
            tc.tile_pool(name="stage", bufs=6) as stage,
            tc.tile_pool(name="stage16", bufs=4) as stage16,
            tc.tile_pool(name="junk", bufs=2) as junkp,
            tc.tile_pool(name="rhs", bufs=3) as rhsp,
            tc.tile_pool(name="exps", bufs=5) as expp,
            tc.tile_pool(name="small", bufs=6) as smallp,
            tc.tile_pool(name="colpp", bufs=2) as colpp,
            tc.tile_pool(name="dscr", bufs=3, space="DRAM") as dramp,
            tc.tile_pool(name="ps_s", bufs=4, space="PSUM") as ps_s,
            tc.tile_pool(name="ps_t", bufs=2, space="PSUM") as ps_t,
            tc.tile_pool(name="ps_c", bufs=2, space="PSUM") as ps_c,
        ):
            identity = const_pool.tile([P, P], bf16)
            make_identity(nc, identity)
            ones = const_pool.tile([P, 1], bf16)
            nc.vector.memset(ones, 1.0)
            eps_bias = const_pool.tile([P, 1], f32)
            nc.vector.memset(eps_bias, EPS_POISSON)

            # persistent accumulators / stats
            zT1_own = persist.tile([P, K, cfg.S], bf16)
            zT2_own = persist.tile([P, K, cfg.S], bf16)
            rowparts = persist.tile([P, 3 * MT * NT], f32)
            nsq_own = persist.tile([P, 3 * MT], f32)
            dots_own = persist.tile([P, 3 * MT], f32)
            poi = persist.tile([P, 2 * MT], f32)
            scaleA = persist.tile([P, 2 * MT], f32)  # (1/T)/||a|| for f1, f2 own rows

            zT_own = [zT1_own, zT2_own]

            def rsqrt_act(dst, src, n, tag):
                # dst[:, :n] = 1/sqrt(src) = exp(-0.5*ln(src)); Ln and Exp share
                # one activation table set (patched below), so no table thrash.
                l = smallp.tile([P, n], f32, tag=tag)
                nc.scalar.activation(l, src, AF.Ln)
                nc.scalar.activation(dst, l, AF.Exp, scale=-0.5)

            def transpose_rowtile_pe(rb16, zT_dst, t):
                # rb16: [128 rows, D] bf16 row-major -> zT_dst[:, k, t*128:(t+1)*128]
                tps = ps_t.tile([P, K * P], bf16, tag="tps")
                for k in range(K):
                    nc.tensor.transpose(tps[:, ts(k, P)], rb16[:, ts(k, P)], identity)
                nc.any.tensor_copy(
                    out=zT_dst[:, :, ts(t, P)],
                    in_=tps[:].rearrange("p (k c) -> p k c", k=K),
                )

            # ---------------- Phase 0a: own f1/f2 (matmul-critical) ----------------
            own_rf = {}
            for t in range(MT):
                for fi in range(2):
                    rf = stage.tile([P, D], f32, tag="rowf32")
                    nc.sync.dma_start(rf, own_dram[fi][ts(t, P), :])
                    own_rf[(fi, t)] = rf
                    rb = stage16.tile([P, D], bf16, tag="rowbf16")
                    nc.vector.tensor_scalar_mul(rb, rf, 1.0)
                    jt = junkp.tile([P, D], bf16, tag="junk16")
                    nc.vector.scalar_tensor_tensor(
                        out=jt, in0=rb, scalar=1.0, in1=rb,
                        op0=ALU.mult, op1=ALU.mult,
                        accum_out=nsq_own[:, fi * MT + t : fi * MT + t + 1],
                    )
                    transpose_rowtile_pe(rb, zT_own[fi], t)

            # own-row exp scales: (1/T) * rsqrt(nsq) for f1, f2
            recip_own = smallp.tile([P, 2 * MT], f32, tag="recip_own")
            rsqrt_act(recip_own, nsq_own[:, : 2 * MT], 2 * MT, tag="ln_own")
            nc.vector.tensor_scalar_mul(scaleA, recip_own, 1.0 / TEMPERATURE)

            def phase0_tail():
                # f3 norms, raw diagonal dots, poisson partials (independent of
                # the matmul stream; emitted last to fill idle DVE/ACT time)
                for t in range(MT):
                    rfs = []
                    for fi in range(3):
                        rf = stage.tile([P, D], f32, tag="rowf32")
                        nc.sync.dma_start(rf, own_dram[fi][ts(t, P), :])
                        rfs.append(rf)
                    jt = junkp.tile([P, D], bf16, tag="junk16")
                    nc.vector.scalar_tensor_tensor(
                        out=jt, in0=rfs[2], scalar=1.0, in1=rfs[2],
                        op0=ALU.mult, op1=ALU.mult,
                        accum_out=nsq_own[:, 2 * MT + t : 2 * MT + t + 1],
                    )
                    for pi, (ia, ib) in enumerate(((0, 1), (0, 2), (1, 2))):
                        jt = junkp.tile([P, D], bf16, tag="junk16")
                        nc.vector.scalar_tensor_tensor(
                            out=jt, in0=rfs[ia], scalar=1.0, in1=rfs[ib],
                            op0=ALU.mult, op1=ALU.mult,
                            accum_out=dots_own[:, pi * MT + t : pi * MT + t + 1],
                        )
                    it = stage.tile([P, D], f32, tag="rowf32")
                    tt = stage.tile([P, D], f32, tag="rowf32")
                    nc.sync.dma_start(it, inp[ts(t, P), :])
                    nc.sync.dma_start(tt, tgt[ts(t, P), :])
                    lg = stage.tile([P, D], f32, tag="rowf32")
                    nc.scalar.activation(lg, it, AF.Ln, bias=eps_bias[:, :])
                    jt = junkp.tile([P, D], bf16, tag="junk16")
                    nc.vector.scalar_tensor_tensor(
                        out=jt, in0=tt, scalar=1.0, in1=lg,
                        op0=ALU.mult, op1=ALU.mult,
                        accum_out=poi[:, MT + t : MT + t + 1],
                    )
                    jt2 = junkp.tile([P, D], bf16, tag="junk16")
                    nc.vector.tensor_scalar(
                        out=jt2, in0=it, scalar1=1.0, scalar2=0.0, op0=ALU.mult,
                        op1=ALU.add, accum_out=poi[:, t : t + 1],
                    )

            # ---------------- Phase 1: stream full f2, f3 ----------------
            # b=0 -> f2_full (rhs of pair0), b=1 -> f3_full (rhs of pair1, pair2)
            # Two-pass software pipeline per feature with a lag of LAG tiles:
            #   produce(b, nt): load f32 rows, squared norms, rsqrt, normalize
            #                   to bf16, write to DRAM scratch (row-major)
            #   consume(b, nt): one xbar DMA transpose scratch -> zT tiles,
            #                   then the matmul/exp/colsum block.
            # This keeps the sync DMA FIFO free of long produce->consume chains
            # so the transposes prefetch ahead of the PE stream.
            full_dram = [f2f, f3f]
            partners_of = [[(0, 0)], [(1, 0), (2, 1)]]
            scratch_b = [
                dramp.tile([B, D], bf16, tag=f"scratch{b}", name=f"scratch{b}")
                for b in range(2)
            ]

            def produce(b, nt):
                nsq_nt = smallp.tile([P, ST], f32, tag="small")
                recip = smallp.tile([P, ST], f32, tag="recipnt")
                rf_tiles = []
                for t in range(ST):
                    rf = stage.tile([P, D], f32, tag="rowf32")
                    nc.sync.dma_start(rf, full_dram[b][nt * NTC + t * P : nt * NTC + (t + 1) * P, :])
                    rf_tiles.append(rf)
                    slot = nsq_nt[:, t : t + 1]
                    if t % 2 == 0:
                        jt = junkp.tile([P, D], bf16, tag="junk16")
                        nc.vector.scalar_tensor_tensor(
                            out=jt, in0=rf, scalar=1.0, in1=rf,
                            op0=ALU.mult, op1=ALU.mult, accum_out=slot,
                        )
                    else:
                        jt = junkp.tile([P, D], bf16, tag="junk16")
                        nc.scalar.activation(jt, rf, AF.Square, accum_out=slot)
                rsqrt_act(recip, nsq_nt, ST, tag="ln_nt")
                for t in range(ST):
                    zrow = stage16.tile([P, D], bf16, tag="rowbf16n")
                    nc.vector.tensor_scalar_mul(zrow, rf_tiles[t], recip[:, t : t + 1])
                    nc.gpsimd.dma_start(scratch_b[b][nt * NTC + t * P : nt * NTC + (t + 1) * P, :], zrow)

            def consume(b, nt):
                zT_rhs = rhsp.tile([P, K, NTC], bf16, tag="zTr")
                nc.sync.dma_start_transpose(
                    zT_rhs[:, :, :], scratch_b[b][nt * NTC : (nt + 1) * NTC, :]
                )
                for (pair, a) in partners_of[b]:
                    exp_tiles = []
                    for m in range(MT):
                        ps = ps_s.tile([P, NTC], f32, tag="ps_s")
                        for k in range(K):
                            nc.tensor.matmul(
                                ps,
                                zT_own[a][:, k, ts(m, P)],
                                zT_rhs[:, k, :],
                                start=(k == 0),
                                stop=(k == K - 1),
                            )
                        es = expp.tile([P, NTC], bf16, tag="exps")
                        slot = (pair * MT + m) * NT + nt
                        nc.scalar.activation(
                            es, ps, AF.Exp,
                            scale=scaleA[:, a * MT + m : a * MT + m + 1],
                            accum_out=rowparts[:, slot : slot + 1],
                        )
                        exp_tiles.append(es)
                    cps = ps_c.tile([1, NTC], f32, tag="ps_c")
                    for m in range(MT):
                        nc.tensor.matmul(
                            cps, ones, exp_tiles[m],
                            start=(m == 0), stop=(m == MT - 1),
                        )
                    colp = colpp.tile([1, NTC], f32, tag="colp")
                    nc.any.tensor_copy(out=colp, in_=cps)
                    nc.gpsimd.dma_start(
                        colparts_d[:, pair * B + nt * NTC : pair * B + (nt + 1) * NTC],
                        colp,
                    )

            LAG = 2
            for b in range(2):
                for nt in range(NT + LAG):
                    if nt < NT:
                        produce(b, nt)
                    if nt - LAG >= 0:
                        consume(b, nt - LAG)

            phase0_tail()

            # ---------------- outputs ----------------
            nc.gpsimd.dma_start(rowparts_d, rowparts)
            nc.gpsimd.dma_start(nsq_d, nsq_own)
            nc.gpsimd.dma_start(dots_d, dots_own)
            nc.gpsimd.dma_start(poi_d, poi)

    nc.compile()
    return nc


def make_in_maps(cfg: Cfg, inputs, targets, feature1, feature2, feature3):
    f32 = np.float32
    ac = np.ascontiguousarray
    maps = []
    for c in range(cfg.n_cores):
        sl = slice(c * cfg.S, (c + 1) * cfg.S)
        maps.append({
            "f1_own": ac(feature1[sl], dtype=f32),
            "f2_own": ac(feature2[sl], dtype=f32),
            "f3_own": ac(feature3[sl], dtype=f32),
            "f2_full": ac(feature2, dtype=f32),
            "f3_full": ac(feature3, dtype=f32),
            "inp_own": ac(inputs[sl], dtype=f32),
            "tgt_own": ac(targets[sl], dtype=f32),
        })
    return maps


def combine_results(cfg: Cfg, per_core):
    """per_core: list of dicts with rowparts/colparts/nsq_own/dots_own/poi."""
    B, MT, NT, S = cfg.B, cfg.MT, cfg.NT, cfg.S
    nsq = np.zeros((3, B), np.float64)
    dots = np.zeros((3, B), np.float64)
    rowsum = np.zeros((3, B), np.float64)
    colsum = np.zeros((3, B), np.float64)
    poi_in = 0.0
    poi_tl = 0.0
    for c, r in enumerate(per_core):
        rp = np.asarray(r["rowparts"], np.float64)      # [128, 3*MT*NT]
        cp = np.asarray(r["colparts"], np.float64)[0]   # [3*B]
        nq = np.asarray(r["nsq_own"], np.float64)       # [128, 3*MT]
        dt_ = np.asarray(r["dots_own"], np.float64)
        po = np.asarray(r["poi"], np.float64)           # [128, 2*MT]
        for fi in range(3):
            for t in range(MT):
                nsq[fi, c * S + t * P : c * S + (t + 1) * P] = nq[:, fi * MT + t]
        for pi in range(3):
            for m in range(MT):
                rows = slice(c * S + m * P, c * S + (m + 1) * P)
                dots[pi, rows] = dt_[:, pi * MT + m]
                rowsum[pi, rows] = rp[:, (pi * MT + m) * NT : (pi * MT + m + 1) * NT].sum(axis=1)
            colsum[pi] += cp[pi * B : (pi + 1) * B]
        poi_in += po[:, :MT].sum()
        poi_tl += po[:, MT:].sum()

    na = np.sqrt(nsq)  # [3, B]
    pairs = ((0, 1), (0, 2), (1, 2))
    closs = 0.0
    for pi, (ia, ib) in enumerate(pairs):
        simdiag = dots[pi] / (na[ia] * na[ib])
        loss_i = np.mean(np.log(rowsum[pi]) - simdiag / TEMPERATURE)
        loss_j = np.mean(np.log(colsum[pi]) - simdiag / TEMPERATURE)
        closs += 0.5 * (loss_i + loss_j)
    closs /= 3.0
    p_loss = (poi_in - poi_tl) / (cfg.B * cfg.D)
    total = p_loss + closs
    return (
        np.float32(total),
        np.float32(p_loss),
        np.float32(closs),
    )


_CACHE = {}


def _get_compiled(cfg: Cfg):
    key = (cfg.B, cfg.D, cfg.n_cores, cfg.NTC)
    if key not in _CACHE:
        _CACHE[key] = build_bass(cfg)
    return _CACHE[key]


def kernel(inputs, targets, feature1, feature2, feature3):
    from concourse.bass_utils import run_bass_kernel_spmd

    cfg = Cfg(B=inputs.shape[0], D=inputs.shape[1], n_cores=8, ntc=512)
    nc = _get_compiled(cfg)
    in_maps = make_in_maps(cfg, inputs, targets, feature1, feature2, feature3)
    res = run_bass_kernel_spmd(nc, in_maps, core_ids=list(range(cfg.n_cores)))
    return combine_results(cfg, res.results)


if __name__ == "__main__":
    # smoke test on hardware with full shapes
    rng = np.random.default_rng(0)
    B, D = 4096, 1024
    ins = {
        "inputs": rng.random((B, D), np.float32),
        "targets": rng.random((B, D), np.float32),
        "feature1": rng.standard_normal((B, D), np.float32),
        "feature2": rng.standard_normal((B, D), np.float32),
        "feature3": rng.standard_normal((B, D), np.float32),
    }
    out = kernel(**ins)
    print(out)



# revision 10
# speedup vs baseline: 1.3163x; 1.3163x over previous
"""Trainium2 Bass kernel for nn_CombinedLoss (Poisson + 3-way pairwise CLIP loss).

Strategy (8 NeuronCores, SPMD, no collectives):
  - Row-shard the batch: core c owns rows [c*512, (c+1)*512) of every tensor.
  - Features are fed to the device pre-TRANSPOSED (raw values, fp8-e4m3 for
    the GEMM operands, bf16 row-major own-slices for norms/diagonals), so the
    kernel never transposes on-chip: no DRAM scratch roundtrip, no PE
    transposes, no DMA-transpose.  All real math (norms, normalize, GEMM,
    softmax sums) stays on device.
  - Main similarity matmuls run in fp8 DoubleRow perf mode (2 k-tiles of 128
    per instruction): lhsT = raw own-row slices of the transposed features,
    rhs = column-normalized (x16) fp8 tiles.  The lhs 1/||a|| normalization,
    the 1/16 rhs prescale and the 1/T all fold into the exp's per-partition
    scale on the Scalar engine.
  - rhs column norms: elementwise squares (split across DVE and GpSimd so
    neither becomes critical) -> fp8, columnwise-summed with fp8 DoubleRow
    ones-matmuls, rsqrt via Ln/Exp (scale=-0.5, bias=ln 16) on ACT, broadcast
    across partitions with a rank-1 ones (x) recip matmul into PSUM, then one
    multiply pass produces the scaled fp8 rhs.
  - Row-wise sum(exp) via the activation's accum_out; column-wise sum(exp)
    via fp8 DoubleRow ones-matmuls over the fp8 exp tiles.
  - Diagonal similarities + row norms from bf16 own-row slices (f32
    accumulators), Poisson partials likewise; host does the O(B) combine.
"""

import math
import os
import sys

import numpy as np

sys.path.insert(0, "/opt/trn_rl_repo")

import ml_dtypes

P = 128
TEMPERATURE = 0.5
EPS_POISSON = 1e-8
RHS_PRESCALE = 16.0  # normalized rhs columns are scaled by this before fp8


class Cfg:
    def __init__(self, B=4096, D=1024, n_cores=8, ntc=512):
        self.B = B          # batch
        self.D = D          # feature dim
        self.n_cores = n_cores
        self.S = B // n_cores      # own rows per core
        self.MT = self.S // P      # M tiles (own rows / 128)
        self.K = D // P            # contraction tiles
        self.NTC = ntc             # columns per rhs tile
        self.NT = B // ntc         # number of rhs tiles
        assert B % n_cores == 0 and self.S % P == 0 and D % P == 0 and B % ntc == 0
        assert self.K % 2 == 0, "DoubleRow needs an even number of k tiles"


def _patch_act_tables():
    """Make Bacc's act-table pass pick `natural_log_exp_and_others` for both
    Exp and Ln (they otherwise land in two different sets, and alternating
    Ln/Exp calls reload the 2.7us activation tables every tile)."""
    import functools

    import concourse.hw_specs as hw_specs

    if getattr(hw_specs, "_act_tables_patched", False):
        return
    orig = hw_specs.get_activation_tables

    @functools.cache
    def patched(module_arch):
        tabs = dict(orig(module_arch))
        names = list(tabs.keys())
        if "natural_log_exp_and_others" in tabs:
            combined = tabs["natural_log_exp_and_others"]
            for name in names:
                if name == "natural_log_exp_and_others":
                    break
                if tabs[name] & combined:
                    tabs[name] = tabs[name] - combined
        return tabs

    hw_specs.get_activation_tables = patched
    import concourse.bacc as bacc_mod

    if hasattr(bacc_mod, "get_activation_tables"):
        bacc_mod.get_activation_tables = patched
    hw_specs._act_tables_patched = True


def build_bass(cfg: Cfg):
    """Build the single-core Bass program (same program for all SPMD cores)."""
    import concourse.bacc as bacc
    import concourse.bass as bass
    import concourse.mybir as mybir
    import concourse.tile as tile

    _patch_act_tables()

    f32 = mybir.dt.float32
    bf16 = mybir.dt.bfloat16
    fp8 = mybir.dt.float8e4
    AF = mybir.ActivationFunctionType
    ALU = mybir.AluOpType
    PM = mybir.MatmulPerfMode
    ts = bass.ts

    B, D, K, MT, NT, NTC, S = cfg.B, cfg.D, cfg.K, cfg.MT, cfg.NT, cfg.NTC, cfg.S

    nc = bacc.Bacc(
        "TRN2",
        target_bir_lowering=False,
        debug=False,
        enable_asserts=False,
        num_devices=cfg.n_cores,
    )

    # ---- IO ----
    # transposed raw fp8 operands for the PE
    f1t_own = nc.dram_tensor("f1t_own", [D, S], fp8, kind="ExternalInput").ap()
    f2t_own = nc.dram_tensor("f2t_own", [D, S], fp8, kind="ExternalInput").ap()
    f2t_full = nc.dram_tensor("f2t_full", [D, B], fp8, kind="ExternalInput").ap()
    f3t_full = nc.dram_tensor("f3t_full", [D, B], fp8, kind="ExternalInput").ap()
    # row-major bf16 own slices for norms / diagonal dots / poisson
    f1o = nc.dram_tensor("f1_own", [S, D], bf16, kind="ExternalInput").ap()
    f2o = nc.dram_tensor("f2_own", [S, D], bf16, kind="ExternalInput").ap()
    f3o = nc.dram_tensor("f3_own", [S, D], bf16, kind="ExternalInput").ap()
    inp = nc.dram_tensor("inp_own", [S, D], bf16, kind="ExternalInput").ap()
    tgt = nc.dram_tensor("tgt_own", [S, D], bf16, kind="ExternalInput").ap()

    rowparts_d = nc.dram_tensor("rowparts", [P, 3 * MT * NT], f32, kind="ExternalOutput").ap()
    colparts_d = nc.dram_tensor("colparts", [1, 3 * B], f32, kind="ExternalOutput").ap()
    nsq_d = nc.dram_tensor("nsq_own", [P, 3 * MT], f32, kind="ExternalOutput").ap()
    dots_d = nc.dram_tensor("dots_own", [P, 3 * MT], f32, kind="ExternalOutput").ap()
    poi_d = nc.dram_tensor("poi", [P, 2 * MT], f32, kind="ExternalOutput").ap()

    own_dram = [f1o, f2o, f3o]
    ownT_dram = [f1t_own, f2t_own]
    full_dram = [f2t_full, f3t_full]
    # feature index 0 (f2_full) is the rhs of pair 0 with lhs f1 (a=0);
    # feature index 1 (f3_full) is the rhs of pairs 1 (lhs f1) and 2 (lhs f2).
    partners_of = [[(0, 0)], [(1, 0), (2, 1)]]

    with tile.TileContext(nc) as tc:
        with (
            tc.tile_pool(name="const", bufs=1) as const_pool,
            tc.tile_pool(name="persist", bufs=1) as persist,
            tc.tile_pool(name="raw", bufs=2) as rawp,       # [P,K,B] fp8 per feature
            tc.tile_pool(name="sq", bufs=1) as sqp,         # [P,K,B] fp8 squares
            tc.tile_pool(name="stage", bufs=6) as stage,    # [P,D] bf16 row staging
            tc.tile_pool(name="junk", bufs=2) as junkp,
            tc.tile_pool(name="exps", bufs=3) as expp,      # [P,MT,NTC] fp8 exp tiles
            tc.tile_pool(name="small", bufs=3) as smallp,
            tc.tile_pool(name="colpp", bufs=2) as colpp,
            tc.tile_pool(name="bcp", bufs=2) as bcp,
            tc.tile_pool(name="dscr", bufs=3, space="DRAM") as dramp,
            tc.tile_pool(name="ps_s", bufs=6, space="PSUM") as ps_s,
            tc.tile_pool(name="ps_sm", bufs=2, space="PSUM") as ps_sm,
        ):
            # dual-fp8 ldweights wants >=16B-aligned even steps between the
            # two k planes, so pad the ones column out to 16 bytes
            ones2_fp8_pad = const_pool.tile([P, 2, 16], fp8)
            nc.vector.memset(ones2_fp8_pad, 1.0)
            ones2_fp8 = ones2_fp8_pad[:, :, 0:1]
            ones1_fp8 = const_pool.tile([P, 1], fp8)
            nc.vector.memset(ones1_fp8, 1.0)
            eps_bias = const_pool.tile([P, 1], f32)
            nc.vector.memset(eps_bias, EPS_POISSON)
            ln16_bias = const_pool.tile([1, 1], f32)
            nc.vector.memset(ln16_bias, math.log(RHS_PRESCALE))

            # persistent accumulators / stats
            zT1_own = persist.tile([P, K, S], fp8)
            zT2_own = persist.tile([P, K, S], fp8)
            rowparts = persist.tile([P, 3 * MT * NT], f32)
            nsq_own = persist.tile([P, 3 * MT], f32)
            dots_own = persist.tile([P, 3 * MT], f32)
            poi = persist.tile([P, 2 * MT], f32)
            scaleA = persist.tile([P, 2 * MT], f32)  # 1/(16*T*||a||) for f1, f2

            zT_own = [zT1_own, zT2_own]

            def rsqrt_act(dst, src, n, tag, bias=None):
                # dst[:, :n] = exp(-0.5*ln(src) + bias) = e^bias/sqrt(src)
                l = smallp.tile([dst.shape[0], n], f32, tag=tag)
                nc.scalar.activation(l, src, AF.Ln)
                if bias is None:
                    nc.scalar.activation(dst, l, AF.Exp, scale=-0.5)
                else:
                    nc.scalar.activation(dst, l, AF.Exp, scale=-0.5, bias=bias)

            # ---------------- Phase 0a: own transposed fp8 slices ----------------
            for fi in range(2):
                for k in range(K):
                    nc.sync.dma_start(
                        zT_own[fi][:, k, :], ownT_dram[fi][ts(k, P), :]
                    )

            # ---------------- Phase 1 emit helpers ----------------
            def prep_feature(b):
                """Load raw fp8 transposed feature b, produce column-normalized
                (x16) fp8 rhs tiles. Returns the scaled tile."""
                raw = rawp.tile([P, K, B], fp8, tag=f"raw{b}")
                sq = sqp.tile([P, K, B], fp8, tag="sq")
                for k in range(K):
                    nc.sync.dma_start(raw[:, k, :], full_dram[b][ts(k, P), :])
                for k in range(K):
                    eng = nc.vector if k % 2 == 0 else nc.gpsimd
                    eng.tensor_tensor(sq[:, k, :], raw[:, k, :], raw[:, k, :], ALU.mult)
                for nt in range(NT):
                    csl = slice(nt * NTC, (nt + 1) * NTC)
                    ps_n = ps_sm.tile([1, NTC], f32, tag="ps_small")
                    for i in range(K // 2):
                        nc.tensor.matmul(
                            ps_n,
                            ones2_fp8,
                            sq[:, 2 * i : 2 * i + 2, csl],
                            start=(i == 0),
                            stop=(i == K // 2 - 1),
                            perf_mode=PM.DoubleRow,
                        )
                    recip = smallp.tile([1, NTC], bf16, tag="recip_nt")
                    rsqrt_act(recip, ps_n, NTC, tag="ln_nt", bias=ln16_bias[:, :])
                    rd = dramp.tile([1, NTC], bf16, tag="recip_dram", name=f"recipd{b}_{nt}")
                    nc.gpsimd.dma_start(rd, recip)
                    bc = bcp.tile([P, NTC], bf16, tag="bc")
                    nc.sync.dma_start(bc, rd.to_broadcast((P, NTC)))
                    # in-place column scale: raw becomes the normalized rhs
                    eng = nc.vector if nt % 2 == 1 else nc.gpsimd
                    eng.tensor_tensor(
                        raw[:, :, csl],
                        raw[:, :, csl],
                        bc.unsqueeze(1).to_broadcast((P, K, NTC)),
                        ALU.mult,
                    )
                return raw

            def matmul_feature(b, scaled):
                """All pair matmuls + exp + row/col sums against rhs feature b."""
                for (pair, a) in partners_of[b]:
                    for nt in range(NT):
                        csl = slice(nt * NTC, (nt + 1) * NTC)
                        es = expp.tile([P, MT, NTC], fp8, tag="exps")
                        for m in range(MT):
                            ps = ps_s.tile([P, NTC], f32, tag="ps_s")
                            for i in range(K // 2):
                                nc.tensor.matmul(
                                    ps,
                                    zT_own[a][:, 2 * i : 2 * i + 2, ts(m, P)],
                                    scaled[:, 2 * i : 2 * i + 2, csl],
                                    start=(i == 0),
                                    stop=(i == K // 2 - 1),
                                    perf_mode=PM.DoubleRow,
                                )
                            slot = (pair * MT + m) * NT + nt
                            nc.scalar.activation(
                                es[:, m, :], ps, AF.Exp,
                                scale=scaleA[:, a * MT + m : a * MT + m + 1],
                                accum_out=rowparts[:, slot : slot + 1],
                            )
                        cps = ps_sm.tile([1, NTC], f32, tag="ps_small")
                        if MT % 2 == 0:
                            for i in range(MT // 2):
                                nc.tensor.matmul(
                                    cps, ones2_fp8, es[:, 2 * i : 2 * i + 2, :],
                                    start=(i == 0), stop=(i == MT // 2 - 1),
                                    perf_mode=PM.DoubleRow,
                                )
                        else:
                            for m in range(MT):
                                nc.tensor.matmul(
                                    cps, ones1_fp8, es[:, m, :],
                                    start=(m == 0), stop=(m == MT - 1),
                                )
                        colp = colpp.tile([1, NTC], f32, tag="colp")
                        nc.any.tensor_copy(out=colp, in_=cps)
                        nc.gpsimd.dma_start(
                            colparts_d[:, pair * B + nt * NTC : pair * B + (nt + 1) * NTC],
                            colp,
                        )

            def phase0_scales():
                # f1/f2 own-row squared norms -> exp scales (gates the first exp)
                for t in range(MT):
                    for fi in range(2):
                        rf = stage.tile([P, D], bf16, tag="rowbf")
                        nc.sync.dma_start(rf, own_dram[fi][ts(t, P), :])
                        jt = junkp.tile([P, D], bf16, tag="junk16")
                        nc.vector.scalar_tensor_tensor(
                            out=jt, in0=rf, scalar=1.0, in1=rf,
                            op0=ALU.mult, op1=ALU.mult,
                            accum_out=nsq_own[:, fi * MT + t : fi * MT + t + 1],
                        )
                # own-row exp scales: 1/(16*T*||a||) for f1, f2
                recip_own = smallp.tile([P, 2 * MT], f32, tag="recip_own")
                rsqrt_act(recip_own, nsq_own[:, : 2 * MT], 2 * MT, tag="ln_own")
                nc.vector.tensor_scalar_mul(
                    scaleA, recip_own, 1.0 / (TEMPERATURE * RHS_PRESCALE)
                )

            def phase0_tail():
                # f3 norms, diagonal dots, poisson partials (gates nothing;
                # fills DVE/ACT while the PE drains the last matmuls)
                for t in range(MT):
                    rfs = []
                    for fi in range(3):
                        rf = stage.tile([P, D], bf16, tag="rowbf")
                        nc.sync.dma_start(rf, own_dram[fi][ts(t, P), :])
                        rfs.append(rf)
                    jt = junkp.tile([P, D], bf16, tag="junk16")
                    nc.scalar.activation(
                        jt, rfs[2], AF.Square,
                        accum_out=nsq_own[:, 2 * MT + t : 2 * MT + t + 1],
                    )
                    for pi, (ia, ib) in enumerate(((0, 1), (0, 2), (1, 2))):
                        jt = junkp.tile([P, D], bf16, tag="junk16")
                        nc.vector.scalar_tensor_tensor(
                            out=jt, in0=rfs[ia], scalar=1.0, in1=rfs[ib],
                            op0=ALU.mult, op1=ALU.mult,
                            accum_out=dots_own[:, pi * MT + t : pi * MT + t + 1],
                        )
                    it = stage.tile([P, D], bf16, tag="rowbf")
                    tt = stage.tile([P, D], bf16, tag="rowbf")
                    nc.sync.dma_start(it, inp[ts(t, P), :])
                    nc.sync.dma_start(tt, tgt[ts(t, P), :])
                    lg = stage.tile([P, D], bf16, tag="rowbf")
                    nc.scalar.activation(lg, it, AF.Ln, bias=eps_bias[:, :])
                    jt = junkp.tile([P, D], bf16, tag="junk16")
                    nc.vector.scalar_tensor_tensor(
                        out=jt, in0=tt, scalar=1.0, in1=lg,
                        op0=ALU.mult, op1=ALU.mult,
                        accum_out=poi[:, MT + t : MT + t + 1],
                    )
                    jt2 = junkp.tile([P, D], bf16, tag="junk16")
                    nc.scalar.activation(
                        jt2, it, AF.Copy, accum_out=poi[:, t : t + 1],
                    )

            # ---------------- emission order ----------------
            phase0_scales()
            scaled2 = prep_feature(0)
            matmul_feature(0, scaled2)
            scaled3 = prep_feature(1)
            matmul_feature(1, scaled3)
            phase0_tail()

            # ---------------- outputs ----------------
            nc.gpsimd.dma_start(rowparts_d, rowparts)
            nc.gpsimd.dma_start(nsq_d, nsq_own)
            nc.gpsimd.dma_start(dots_d, dots_own)
            nc.gpsimd.dma_start(poi_d, poi)

    nc.compile()
    return nc


def make_in_maps(cfg: Cfg, inputs, targets, feature1, feature2, feature3):
    bf16 = ml_dtypes.bfloat16
    fp8 = ml_dtypes.float8_e4m3
    ac = np.ascontiguousarray

    f1t = ac(feature1.T).astype(fp8)
    f2t = ac(feature2.T).astype(fp8)
    f3t = ac(feature3.T).astype(fp8)
    f1b = np.asarray(feature1).astype(bf16)
    f2b = np.asarray(feature2).astype(bf16)
    f3b = np.asarray(feature3).astype(bf16)
    inb = np.asarray(inputs).astype(bf16)
    tgb = np.asarray(targets).astype(bf16)

    maps = []
    for c in range(cfg.n_cores):
        sl = slice(c * cfg.S, (c + 1) * cfg.S)
        maps.append({
            "f1t_own": ac(f1t[:, sl]),
            "f2t_own": ac(f2t[:, sl]),
            "f2t_full": f2t,
            "f3t_full": f3t,
            "f1_own": ac(f1b[sl]),
            "f2_own": ac(f2b[sl]),
            "f3_own": ac(f3b[sl]),
            "inp_own": ac(inb[sl]),
            "tgt_own": ac(tgb[sl]),
        })
    return maps


def combine_results(cfg: Cfg, per_core):
    """per_core: list of dicts with rowparts/colparts/nsq_own/dots_own/poi."""
    B, MT, NT, S = cfg.B, cfg.MT, cfg.NT, cfg.S
    nsq = np.zeros((3, B), np.float64)
    dots = np.zeros((3, B), np.float64)
    rowsum = np.zeros((3, B), np.float64)
    colsum = np.zeros((3, B), np.float64)
    poi_in = 0.0
    poi_tl = 0.0
    for c, r in enumerate(per_core):
        rp = np.asarray(r["rowparts"], np.float64)      # [128, 3*MT*NT]
        cp = np.asarray(r["colparts"], np.float64)[0]   # [3*B]
        nq = np.asarray(r["nsq_own"], np.float64)       # [128, 3*MT]
        dt_ = np.asarray(r["dots_own"], np.float64)
        po = np.asarray(r["poi"], np.float64)           # [128, 2*MT]
        for fi in range(3):
            for t in range(MT):
                nsq[fi, c * S + t * P : c * S + (t + 1) * P] = nq[:, fi * MT + t]
        for pi in range(3):
            for m in range(MT):
                rows = slice(c * S + m * P, c * S + (m + 1) * P)
                dots[pi, rows] = dt_[:, pi * MT + m]
                rowsum[pi, rows] = rp[:, (pi * MT + m) * NT : (pi * MT + m + 1) * NT].sum(axis=1)
            colsum[pi] += cp[pi * B : (pi + 1) * B]
        poi_in += po[:, :MT].sum()
        poi_tl += po[:, MT:].sum()

    na = np.sqrt(nsq)  # [3, B]
    pairs = ((0, 1), (0, 2), (1, 2))
    closs = 0.0
    for pi, (ia, ib) in enumerate(pairs):
        simdiag = dots[pi] / (na[ia] * na[ib])
        loss_i = np.mean(np.log(rowsum[pi]) - simdiag / TEMPERATURE)
        loss_j = np.mean(np.log(colsum[pi]) - simdiag / TEMPERATURE)
        closs += 0.5 * (loss_i + loss_j)
    closs /= 3.0
    p_loss = (poi_in - poi_tl) / (cfg.B * cfg.D)
    total = p_loss + closs
    return (
        np.float32(total),
        np.float32(p_loss),
        np.float32(closs),
    )


_CACHE = {}


def _get_compiled(cfg: Cfg):
    key = (cfg.B, cfg.D, cfg.n_cores, cfg.NTC)
    if key not in _CACHE:
        _CACHE[key] = build_bass(cfg)
    return _CACHE[key]


def kernel(inputs, targets, feature1, feature2, feature3):
    from concourse.bass_utils import run_bass_kernel_spmd

    cfg = Cfg(B=inputs.shape[0], D=inputs.shape[1], n_cores=8, ntc=512)
    nc = _get_compiled(cfg)
    in_maps = make_in_maps(cfg, inputs, targets, feature1, feature2, feature3)
    res = run_bass_kernel_spmd(nc, in_maps, core_ids=list(range(cfg.n_cores)))
    return combine_results(cfg, res.results)


if __name__ == "__main__":
    # smoke test on hardware with full shapes
    rng = np.random.default_rng(0)
    B, D = 4096, 1024
    ins = {
        "inputs": rng.random((B, D)).astype(np.float32),
        "targets": rng.random((B, D)).astype(np.float32),
        "feature1": rng.standard_normal((B, D)).astype(np.float32),
        "feature2": rng.standard_normal((B, D)).astype(np.float32),
        "feature3": rng.standard_normal((B, D)).astype(np.float32),
    }
    out = kernel(**ins)
    print(out)


# revision 12
# speedup vs baseline: 1.8517x; 1.4068x over previous
"""Trainium2 Bass kernel for nn_CombinedLoss (Poisson + 3-way pairwise CLIP loss).

Strategy (8 NeuronCores, SPMD, no collectives):
  - Row-shard the batch: core c owns rows [c*512, (c+1)*512) of every tensor.
  - Features are fed to the device pre-TRANSPOSED (raw values, fp8-e4m3 for
    the full features, bf16 for the own column-slices) plus bf16 row-major
    copies for norms/diagonals, so the kernel never transposes data tiles
    on-chip.  All real math (norms, normalize, GEMM, softmax sums) stays on
    device.
  - The similarity matrices are computed TRANSPOSED: for each pair (a,b),
    S^T[n,m] = f_b[n] . f_a[m] with lhsT = raw fp8 128-row blocks of the full
    transposed partner feature and the moving rhs = the core's own 512
    columns, in fp8 DoubleRow perf mode (2 k-tiles of 128 per instruction).
    In this orientation the partner-row normalization 1/||b_n|| (and the 1/T,
    and the fp8 prescale) ride the exp's native per-partition scale, and the
    own-row normalization 16/||a_m|| is folded into the own fp8 tiles once.
    No O(B*D) elementwise pass over the full features is needed at all.
  - Partner row norms come from bf16 row-major tiles via squared-sum
    accumulate on the Vector engine (2-byte fast path + free reduction).
  - Column sums of S (= over the core's own rows, for loss_j) come free via
    the exp activation's accum_out; row sums of S (for loss_i) via fp8
    DoubleRow ones-matmuls over the fp8 exp tiles, PSUM-accumulated across
    the partner blocks.
  - Diagonal similarities + own norms from bf16 own-row slices (f32
    accumulators), Poisson partials likewise; host does the O(B) combine.
"""

import math
import os
import sys

import numpy as np

sys.path.insert(0, "/opt/trn_rl_repo")

import ml_dtypes

P = 128
TEMPERATURE = 0.5
EPS_POISSON = 1e-8
OWN_PRESCALE = 16.0  # normalized own columns are scaled by this before fp8


class Cfg:
    def __init__(self, B=4096, D=1024, n_cores=8, ntc=512):
        self.B = B          # batch
        self.D = D          # feature dim
        self.n_cores = n_cores
        self.S = B // n_cores      # own rows per core
        self.MT = self.S // P      # M tiles (own rows / 128)
        self.K = D // P            # contraction tiles
        self.NB = B // P           # partner row blocks of 128
        self.NTC = ntc             # unused (kept for cache key compat)
        assert B % n_cores == 0 and self.S % P == 0 and D % P == 0
        assert self.K % 2 == 0, "DoubleRow needs an even number of k tiles"
        assert self.NB % 2 == 0, "DoubleRow rowsum needs an even block count"


def _patch_act_tables():
    """Make Bacc's act-table pass pick `natural_log_exp_and_others` for both
    Exp and Ln (they otherwise land in two different sets, and alternating
    Ln/Exp calls reload the 2.7us activation tables every tile)."""
    import functools

    import concourse.hw_specs as hw_specs

    if getattr(hw_specs, "_act_tables_patched", False):
        return
    orig = hw_specs.get_activation_tables

    @functools.cache
    def patched(module_arch):
        tabs = dict(orig(module_arch))
        names = list(tabs.keys())
        if "natural_log_exp_and_others" in tabs:
            combined = tabs["natural_log_exp_and_others"]
            for name in names:
                if name == "natural_log_exp_and_others":
                    break
                if tabs[name] & combined:
                    tabs[name] = tabs[name] - combined
        return tabs

    hw_specs.get_activation_tables = patched
    import concourse.bacc as bacc_mod

    if hasattr(bacc_mod, "get_activation_tables"):
        bacc_mod.get_activation_tables = patched
    hw_specs._act_tables_patched = True


def build_bass(cfg: Cfg):
    """Build the single-core Bass program (same program for all SPMD cores)."""
    import concourse.bacc as bacc
    import concourse.bass as bass
    import concourse.mybir as mybir
    import concourse.tile as tile
    from concourse.masks import make_identity

    _patch_act_tables()

    f32 = mybir.dt.float32
    bf16 = mybir.dt.bfloat16
    fp8 = mybir.dt.float8e4
    AF = mybir.ActivationFunctionType
    ALU = mybir.AluOpType
    PM = mybir.MatmulPerfMode
    ts = bass.ts

    B, D, K, MT, NB, S = cfg.B, cfg.D, cfg.K, cfg.MT, cfg.NB, cfg.S

    nc = bacc.Bacc(
        "TRN2",
        target_bir_lowering=False,
        debug=False,
        enable_asserts=False,
        num_devices=cfg.n_cores,
    )

    # ---- IO ----
    # transposed raw fp8 full features (lhsT source for the PE)
    f2t_full = nc.dram_tensor("f2t_full", [D, B], fp8, kind="ExternalInput").ap()
    f3t_full = nc.dram_tensor("f3t_full", [D, B], fp8, kind="ExternalInput").ap()
    # transposed raw bf16 own column-slices (scaled to fp8 on device)
    f1t_own = nc.dram_tensor("f1t_own", [D, S], bf16, kind="ExternalInput").ap()
    f2t_own = nc.dram_tensor("f2t_own", [D, S], bf16, kind="ExternalInput").ap()
    # row-major bf16 full partner features (for row norms)
    f2_rm = nc.dram_tensor("f2_rm", [B, D], bf16, kind="ExternalInput").ap()
    f3_rm = nc.dram_tensor("f3_rm", [B, D], bf16, kind="ExternalInput").ap()
    # row-major bf16 own slices for norms / diagonal dots / poisson
    f1o = nc.dram_tensor("f1_own", [S, D], bf16, kind="ExternalInput").ap()
    f2o = nc.dram_tensor("f2_own", [S, D], bf16, kind="ExternalInput").ap()
    f3o = nc.dram_tensor("f3_own", [S, D], bf16, kind="ExternalInput").ap()
    inp = nc.dram_tensor("inp_own", [S, D], bf16, kind="ExternalInput").ap()
    tgt = nc.dram_tensor("tgt_own", [S, D], bf16, kind="ExternalInput").ap()

    # rowparts[0, pair*S + m] = sum_n exp(logit[m, n]) for own row m
    rowparts_d = nc.dram_tensor("rowparts", [1, 3 * S], f32, kind="ExternalOutput").ap()
    # colparts[p, pair*NB + nb] = sum over own rows of exp(logit[:, nb*128+p])
    colparts_d = nc.dram_tensor("colparts", [P, 3 * NB], f32, kind="ExternalOutput").ap()
    nsq_d = nc.dram_tensor("nsq_own", [P, 3 * MT], f32, kind="ExternalOutput").ap()
    dots_d = nc.dram_tensor("dots_own", [P, 3 * MT], f32, kind="ExternalOutput").ap()
    poi_d = nc.dram_tensor("poi", [P, 2 * MT], f32, kind="ExternalOutput").ap()

    own_dram = [f1o, f2o, f3o]
    ownT_dram = [f1t_own, f2t_own]
    full_dram = [f2t_full, f3t_full]
    rm_dram = [f2_rm, f3_rm]
    # feature index 0 (f2) partners pair 0 with own f1 (a=0);
    # feature index 1 (f3) partners pairs 1 (own f1) and 2 (own f2).
    partners_of = [[(0, 0)], [(1, 0), (2, 1)]]

    with tile.TileContext(nc) as tc:
        with (
            tc.tile_pool(name="const", bufs=1) as const_pool,
            tc.tile_pool(name="persist", bufs=1) as persist,
            tc.tile_pool(name="raw", bufs=2) as rawp,       # [P,K,B] fp8 per feature
            tc.tile_pool(name="ownT", bufs=2) as ownTp,     # [P,K,S] bf16 staging
            tc.tile_pool(name="stage", bufs=6) as stage,    # [P,D] bf16 row staging
            tc.tile_pool(name="junk", bufs=2) as junkp,
            tc.tile_pool(name="exps", bufs=3) as expp,      # [P,2,S] fp8 exp tiles
            tc.tile_pool(name="small", bufs=3) as smallp,
            tc.tile_pool(name="rowout", bufs=1) as rowoutp,
            tc.tile_pool(name="dscr", bufs=2, space="DRAM") as dramp,
            tc.tile_pool(name="ps_s", bufs=4, space="PSUM") as ps_s,
            tc.tile_pool(name="ps_row", bufs=1, space="PSUM") as ps_row,
            tc.tile_pool(name="ps_t", bufs=1, space="PSUM") as ps_t,
        ):
            # dual-fp8 ldweights wants >=16B-aligned even steps between the
            # two k planes, so pad the ones column out to 16 bytes
            ones2_fp8_pad = const_pool.tile([P, 2, 16], fp8)
            nc.vector.memset(ones2_fp8_pad, 1.0)
            ones2_fp8 = ones2_fp8_pad[:, :, 0:1]
            ones1_fp8 = const_pool.tile([P, 1], fp8)
            nc.vector.memset(ones1_fp8, 1.0)
            eps_bias = const_pool.tile([P, 1], f32)
            nc.vector.memset(eps_bias, EPS_POISSON)
            ln16_bias = const_pool.tile([P, 1], f32)
            nc.vector.memset(ln16_bias, math.log(OWN_PRESCALE))
            # exp-scale rsqrt bias: recip_rhs = exp(-ln(nsq)/2 + ln(1/(16*T)))
            lnsc_bias = const_pool.tile([P, 1], f32)
            nc.vector.memset(lnsc_bias, math.log(1.0 / (OWN_PRESCALE * TEMPERATURE)))
            identity = const_pool.tile([P, P], bf16)
            make_identity(nc, identity)

            # persistent accumulators / stats
            zT1_own = persist.tile([P, K, S], fp8)
            zT2_own = persist.tile([P, K, S], fp8)
            colgrid = persist.tile([P, 3 * NB], f32)
            nsq_own = persist.tile([P, 3 * MT], f32)
            dots_own = persist.tile([P, 3 * MT], f32)
            poi = persist.tile([P, 2 * MT], f32)
            nsq_rhs = persist.tile([P, 2 * NB], f32)
            recip_rhs = persist.tile([P, 2 * NB], f32)  # 1/(16*T*||b_n||)

            zT_own = [zT1_own, zT2_own]
            rowout = rowoutp.tile([1, 3 * S], f32)

            def rsqrt_act(dst, src, n, tag, bias=None):
                # dst[:, :n] = exp(-0.5*ln(src) + bias) = e^bias/sqrt(src)
                l = smallp.tile([dst.shape[0], n], f32, tag=tag)
                nc.scalar.activation(l, src, AF.Ln)
                if bias is None:
                    nc.scalar.activation(dst, l, AF.Exp, scale=-0.5)
                else:
                    nc.scalar.activation(dst, l, AF.Exp, scale=-0.5, bias=bias)

            # ---------------- Phase 0a: own lhs scales + scaled own fp8 ----------------
            def phase0_scales():
                # f1/f2 own-row squared norms (bf16 rows, f32 accum)
                for t in range(MT):
                    for fi in range(2):
                        rf = stage.tile([P, D], bf16, tag="rowbf")
                        nc.sync.dma_start(rf, own_dram[fi][ts(t, P), :])
                        jt = junkp.tile([P, D], bf16, tag="junk16")
                        nc.vector.scalar_tensor_tensor(
                            out=jt, in0=rf, scalar=1.0, in1=rf,
                            op0=ALU.mult, op1=ALU.mult,
                            accum_out=nsq_own[:, fi * MT + t : fi * MT + t + 1],
                        )
                # recip_own grid [P, 2*MT] = 16/||a||  (bf16 for the transpose)
                recip_own = smallp.tile([P, 2 * MT], bf16, tag="recip_own")
                rsqrt_act(recip_own, nsq_own[:, : 2 * MT], 2 * MT, tag="ln_own",
                          bias=ln16_bias[:, :])
                # transpose to a row vector: grid[p, fi*MT+t] is own row t*128+p,
                # so transpose -> [2*MT, 128] whose flat order per fi is the
                # global own-row index.  PE transpose + copy + DRAM roundtrip,
                # then broadcast-read back as [P, S] per feature.
                tps = ps_t.tile([2 * MT, P], bf16, tag="tps")
                nc.tensor.transpose(tps, recip_own, identity)
                rT = smallp.tile([2 * MT, P], bf16, tag="rT")
                nc.any.tensor_copy(out=rT, in_=tps)
                rd = dramp.tile([2, S], bf16, tag="recip_own_dram", name="recip_own_d")
                nc.gpsimd.dma_start(
                    rd.rearrange("f (t p) -> (f t) p", p=P), rT
                )
                for fi in range(2):
                    bc = smallp.tile([P, S], bf16, tag=f"bc_own{fi}")
                    nc.sync.dma_start(bc, rd[fi : fi + 1, :].to_broadcast((P, S)))
                    zb = ownTp.tile([P, K, S], bf16, tag=f"ownT{fi}")
                    for k in range(K):
                        nc.sync.dma_start(zb[:, k, :], ownT_dram[fi][ts(k, P), :])
                    for k in range(K):
                        eng = nc.vector if k % 2 == 0 else nc.gpsimd
                        eng.tensor_tensor(zT_own[fi][:, k, :], zb[:, k, :], bc, ALU.mult)

            # ---------------- partner feature norms ----------------
            def rhs_norms(b):
                for nb in range(NB):
                    rf = stage.tile([P, D], bf16, tag="rowbf")
                    nc.sync.dma_start(rf, rm_dram[b][ts(nb, P), :])
                    jt = junkp.tile([P, D], bf16, tag="junk16")
                    nc.vector.scalar_tensor_tensor(
                        out=jt, in0=rf, scalar=1.0, in1=rf,
                        op0=ALU.mult, op1=ALU.mult,
                        accum_out=nsq_rhs[:, b * NB + nb : b * NB + nb + 1],
                    )
                rsqrt_act(
                    recip_rhs[:, b * NB : (b + 1) * NB],
                    nsq_rhs[:, b * NB : (b + 1) * NB],
                    NB, tag="ln_rhs", bias=lnsc_bias[:, :],
                )

            def load_feature(b):
                raw = rawp.tile([P, K, B], fp8, tag=f"raw{b}")
                for k in range(K):
                    nc.sync.dma_start(raw[:, k, :], full_dram[b][ts(k, P), :])
                return raw

            # ---------------- main matmuls (S^T orientation) ----------------
            def matmul_feature(b, raw):
                for (pair, a) in partners_of[b]:
                    rps = ps_row.tile([1, S], f32, tag=f"rps{pair}")
                    nbp_count = NB // 2
                    for nbp in range(nbp_count):
                        es = expp.tile([P, 2, S], fp8, tag="exps")
                        for j in range(2):
                            nb = 2 * nbp + j
                            ps = ps_s.tile([P, S], f32, tag="ps_s")
                            for i in range(K // 2):
                                nc.tensor.matmul(
                                    ps,
                                    raw[:, 2 * i : 2 * i + 2, ts(nb, P)],
                                    zT_own[a][:, 2 * i : 2 * i + 2, :],
                                    start=(i == 0),
                                    stop=(i == K // 2 - 1),
                                    perf_mode=PM.DoubleRow,
                                )
                            slot = b * NB + nb
                            cslot = pair * NB + nb
                            nc.scalar.activation(
                                es[:, j, :], ps, AF.Exp,
                                scale=recip_rhs[:, slot : slot + 1],
                                accum_out=colgrid[:, cslot : cslot + 1],
                            )
                        nc.tensor.matmul(
                            rps, ones2_fp8, es,
                            start=(nbp == 0), stop=(nbp == nbp_count - 1),
                            perf_mode=PM.DoubleRow,
                            skip_group_check=True,
                        )
                    nc.any.tensor_copy(
                        out=rowout[:, pair * S : (pair + 1) * S], in_=rps
                    )

            def phase0_tail():
                # f3 norms, diagonal dots, poisson partials (gates nothing;
                # fills DVE/ACT while the PE drains the last matmuls)
                for t in range(MT):
                    rfs = []
                    for fi in range(3):
                        rf = stage.tile([P, D], bf16, tag="rowbf")
                        nc.sync.dma_start(rf, own_dram[fi][ts(t, P), :])
                        rfs.append(rf)
                    jt = junkp.tile([P, D], bf16, tag="junk16")
                    nc.scalar.activation(
                        jt, rfs[2], AF.Square,
                        accum_out=nsq_own[:, 2 * MT + t : 2 * MT + t + 1],
                    )
                    for pi, (ia, ib) in enumerate(((0, 1), (0, 2), (1, 2))):
                        jt = junkp.tile([P, D], bf16, tag="junk16")
                        nc.vector.scalar_tensor_tensor(
                            out=jt, in0=rfs[ia], scalar=1.0, in1=rfs[ib],
                            op0=ALU.mult, op1=ALU.mult,
                            accum_out=dots_own[:, pi * MT + t : pi * MT + t + 1],
                        )
                    it = stage.tile([P, D], bf16, tag="rowbf")
                    tt = stage.tile([P, D], bf16, tag="rowbf")
                    nc.sync.dma_start(it, inp[ts(t, P), :])
                    nc.sync.dma_start(tt, tgt[ts(t, P), :])
                    lg = stage.tile([P, D], bf16, tag="rowbf")
                    nc.scalar.activation(lg, it, AF.Ln, bias=eps_bias[:, :])
                    jt = junkp.tile([P, D], bf16, tag="junk16")
                    nc.vector.scalar_tensor_tensor(
                        out=jt, in0=tt, scalar=1.0, in1=lg,
                        op0=ALU.mult, op1=ALU.mult,
                        accum_out=poi[:, MT + t : MT + t + 1],
                    )
                    jt2 = junkp.tile([P, D], bf16, tag="junk16")
                    nc.scalar.activation(
                        jt2, it, AF.Copy, accum_out=poi[:, t : t + 1],
                    )

            # ---------------- emission order ----------------
            phase0_scales()
            rhs_norms(0)
            raw2 = load_feature(0)
            matmul_feature(0, raw2)
            rhs_norms(1)
            raw3 = load_feature(1)
            matmul_feature(1, raw3)
            phase0_tail()

            # ---------------- outputs ----------------
            nc.gpsimd.dma_start(rowparts_d, rowout)
            nc.gpsimd.dma_start(colparts_d, colgrid)
            nc.gpsimd.dma_start(nsq_d, nsq_own)
            nc.gpsimd.dma_start(dots_d, dots_own)
            nc.gpsimd.dma_start(poi_d, poi)

    nc.compile()
    return nc


def make_in_maps(cfg: Cfg, inputs, targets, feature1, feature2, feature3):
    bf16 = ml_dtypes.bfloat16
    fp8 = ml_dtypes.float8_e4m3
    ac = np.ascontiguousarray

    f1t = ac(np.asarray(feature1).T).astype(bf16)
    f2t = ac(np.asarray(feature2).T)
    f3t = ac(np.asarray(feature3).T)
    f2t8 = f2t.astype(fp8)
    f3t8 = f3t.astype(fp8)
    f2tb = f2t.astype(bf16)
    f1b = np.asarray(feature1).astype(bf16)
    f2b = np.asarray(feature2).astype(bf16)
    f3b = np.asarray(feature3).astype(bf16)
    inb = np.asarray(inputs).astype(bf16)
    tgb = np.asarray(targets).astype(bf16)

    maps = []
    for c in range(cfg.n_cores):
        sl = slice(c * cfg.S, (c + 1) * cfg.S)
        maps.append({
            "f2t_full": f2t8,
            "f3t_full": f3t8,
            "f1t_own": ac(f1t[:, sl]),
            "f2t_own": ac(f2tb[:, sl]),
            "f2_rm": f2b,
            "f3_rm": f3b,
            "f1_own": ac(f1b[sl]),
            "f2_own": ac(f2b[sl]),
            "f3_own": ac(f3b[sl]),
            "inp_own": ac(inb[sl]),
            "tgt_own": ac(tgb[sl]),
        })
    return maps


def combine_results(cfg: Cfg, per_core):
    """per_core: list of dicts with rowparts/colparts/nsq_own/dots_own/poi."""
    B, MT, NB, S = cfg.B, cfg.MT, cfg.NB, cfg.S
    nsq = np.zeros((3, B), np.float64)
    dots = np.zeros((3, B), np.float64)
    rowsum = np.zeros((3, B), np.float64)   # per own row m: sum_n exp
    colsum = np.zeros((3, B), np.float64)   # per partner row n: sum_m exp
    poi_in = 0.0
    poi_tl = 0.0
    for c, r in enumerate(per_core):
        rp = np.asarray(r["rowparts"], np.float64)[0]   # [3*S]
        cg = np.asarray(r["colparts"], np.float64)      # [128, 3*NB]
        nq = np.asarray(r["nsq_own"], np.float64)       # [128, 3*MT]
        dt_ = np.asarray(r["dots_own"], np.float64)
        po = np.asarray(r["poi"], np.float64)           # [128, 2*MT]
        for fi in range(3):
            for t in range(MT):
                nsq[fi, c * S + t * P : c * S + (t + 1) * P] = nq[:, fi * MT + t]
        for pi in range(3):
            for t in range(MT):
                rows = slice(c * S + t * P, c * S + (t + 1) * P)
                dots[pi, rows] = dt_[:, pi * MT + t]
            rowsum[pi, c * S : (c + 1) * S] = rp[pi * S : (pi + 1) * S]
            for nb in range(NB):
                colsum[pi, nb * P : (nb + 1) * P] += cg[:, pi * NB + nb]
        poi_in += po[:, :MT].sum()
        poi_tl += po[:, MT:].sum()

    na = np.sqrt(nsq)  # [3, B]
    pairs = ((0, 1), (0, 2), (1, 2))
    closs = 0.0
    for pi, (ia, ib) in enumerate(pairs):
        simdiag = dots[pi] / (na[ia] * na[ib])
        # rowsum[m] = sum_n exp(logit[m,n])  -> softmax over partner (loss_i)
        # colsum[n] = sum_m exp(logit[m,n])  -> softmax over own rows (loss_j)
        loss_i = np.mean(np.log(rowsum[pi]) - simdiag / TEMPERATURE)
        loss_j = np.mean(np.log(colsum[pi]) - simdiag / TEMPERATURE)
        closs += 0.5 * (loss_i + loss_j)
    closs /= 3.0
    p_loss = (poi_in - poi_tl) / (cfg.B * cfg.D)
    total = p_loss + closs
    return (
        np.float32(total),
        np.float32(p_loss),
        np.float32(closs),
    )


_CACHE = {}


def _get_compiled(cfg: Cfg):
    key = (cfg.B, cfg.D, cfg.n_cores, cfg.NTC)
    if key not in _CACHE:
        _CACHE[key] = build_bass(cfg)
    return _CACHE[key]


def kernel(inputs, targets, feature1, feature2, feature3):
    from concourse.bass_utils import run_bass_kernel_spmd

    cfg = Cfg(B=inputs.shape[0], D=inputs.shape[1], n_cores=8, ntc=512)
    nc = _get_compiled(cfg)
    in_maps = make_in_maps(cfg, inputs, targets, feature1, feature2, feature3)
    res = run_bass_kernel_spmd(nc, in_maps, core_ids=list(range(cfg.n_cores)))
    return combine_results(cfg, res.results)


if __name__ == "__main__":
    # smoke test on hardware with full shapes
    rng = np.random.default_rng(0)
    B, D = 4096, 1024
    ins = {
        "inputs": rng.random((B, D)).astype(np.float32),
        "targets": rng.random((B, D)).astype(np.float32),
        "feature1": rng.standard_normal((B, D)).astype(np.float32),
        "feature2": rng.standard_normal((B, D)).astype(np.float32),
        "feature3": rng.standard_normal((B, D)).astype(np.float32),
    }
    out = kernel(**ins)
    print(out)
